# revision 8
# baseline (speedup 1.0000x reference)
"""Trainium2 8-core kernel for nn_AlignedGloveLayer (retrieval 1-NN mismatch loss).

Problem: a = mapped[indexes] ([4096, 256]); d2[k, j] = |a_k - target_j|^2 over
30000 targets; loss = mean over k of (argmin_j d2[k, j] != indexes[k]).

Only the comparison min_j d2 vs d2[:, indexes[k]] matters (sqrt is monotone and
the a2 term is constant per row), so the device computes, per query,
m_k = min_j (b2_j - 2 a_k . t_j). The mismatch decision and the final mean are
assembled on the host, with an exact fp64 fallback for any query whose margin
is within the device-arithmetic error bound (bf16 matmul + fp16 min).

Sharding (2x4 grid): cores 0-3 take 1024 queries each over the first half of
the targets; cores 4-7 take the same query slices over the second half; the
host mins the two halves. Per core, flipped orientation:
  psum[t, q] = sum_d T[t, d] * (-2 a[q, d])   (targets on psum partitions)
  ScalarE:  val16[t, q] = psum + (b2[t] - SHIFT)   (per-partition bias; a few
            chunks go through VectorE tensor_scalar instead to balance load)
  VectorE:  acc16 = min(acc16, val16)              (fp16, 2x mode, 4 rotating accs)
  final:    PE-transpose acc16 -> min-reduce free dim -> m[128, 8]
Targets are padded 30000 -> 30720 = 240*128; padded rows get b2' = 60000.
"""
import os
import sys

for _p in ("/opt/trn_rl_repo", "/root/.axon_site/_ro/trn_rl_repo"):
    if os.path.isdir(_p) and _p not in sys.path:
        sys.path.append(_p)

from contextlib import ExitStack

import ml_dtypes
import numpy as np

NX, NY, D, K = 30000, 30000, 256, 4096
NCORES = 8
P = 128
DC = D // P          # 2 contraction chunks
NQ = 1024            # queries per core (cores c and c+4 share a query slice)
QB = NQ // P         # 8 query blocks
TCH = 240            # total target chunks: 240*128 = 30720 >= 30000
TCHH = TCH // 2      # target chunks per core (half of the targets)
NYP = TCH * P
GROUP = 8            # target chunks per DMA batch
DVE_AFFINE_EVERY = 6 # every Nth chunk's affine goes to VectorE instead of ScalarE
SHIFT = 512.0        # centers val16 in fp16 range
INIT = 60000.0       # min-accumulator init (> any real val')
PADVAL = 60000.0     # padded targets' shifted b2 (never the min)
DELTA = 3.0          # device error bound for host fallback flagging

_CACHE: dict = {}


def _build_nc():
    import concourse.tile as tile
    from concourse import bacc, mybir
    from concourse.masks import make_identity

    nc = bacc.Bacc("TRN2", target_bir_lowering=False)
    at_d = nc.dram_tensor("at", [P, DC, NQ], mybir.dt.bfloat16, kind="ExternalInput")
    tt_d = nc.dram_tensor("tt", [P, TCHH, DC, P], mybir.dt.bfloat16, kind="ExternalInput")
    b2_d = nc.dram_tensor("b2c", [P, TCHH], mybir.dt.float32, kind="ExternalInput")
    m_d = nc.dram_tensor("m", [P, QB], mybir.dt.float32, kind="ExternalOutput")

    with tile.TileContext(nc) as tc:
        with ExitStack() as ctx:
            sb = ctx.enter_context(tc.tile_pool(name="sb", bufs=1))
            stream = ctx.enter_context(tc.tile_pool(name="stream", bufs=6))
            vals = ctx.enter_context(tc.tile_pool(name="vals", bufs=4))
            psum = ctx.enter_context(tc.tile_pool(name="psum", bufs=3, space="PSUM"))
            psum2 = ctx.enter_context(tc.tile_pool(name="psum2", bufs=2, space="PSUM"))

            at = sb.tile([P, DC, NQ], mybir.dt.bfloat16)
            nc.sync.dma_start(at[:], at_d[:])
            b2c = sb.tile([P, TCHH], mybir.dt.float32)
            nc.sync.dma_start(b2c[:], b2_d[:])
            ident = sb.tile([P, P], mybir.dt.float16)
            make_identity(nc, ident)

            NACC = 4     # rotating min accumulators (break DVE dependency chain)
            accs = []
            for i in range(NACC):
                a_t = sb.tile([P, NQ], mybir.dt.float16, tag=f"acc{i}", name=f"acc{i}")
                nc.vector.memset(a_t[:], INIT)
                accs.append(a_t)

            for g in range(TCHH // GROUP):
                tt = stream.tile([P, GROUP, DC, P], mybir.dt.bfloat16, tag="tt")
                nc.sync.dma_start(tt[:], tt_d[:, g * GROUP:(g + 1) * GROUP])
                for j in range(GROUP):
                    t = g * GROUP + j
                    ps = psum.tile([P, NQ], mybir.dt.float32)
                    # matmul N<=512: two free-dim halves, stationary shared per dc
                    for dc in range(DC):
                        for h in range(NQ // 512):
                            nc.tensor.matmul(
                                ps[:, h * 512:(h + 1) * 512],
                                tt[:, j, dc, :],
                                at[:, dc, h * 512:(h + 1) * 512],
                                start=(dc == 0), stop=(dc == DC - 1),
                            )
                    val = vals.tile([P, NQ], mybir.dt.float16, tag="val")
                    if t % DVE_AFFINE_EVERY == DVE_AFFINE_EVERY - 1:
                        # VectorE route (offload ScalarE): same affine
                        nc.vector.tensor_scalar(
                            val[:], ps[:], b2c[:, t:t + 1], None,
                            mybir.AluOpType.add,
                        )
                    else:
                        # ScalarE route: val = psum + b2' (per-partition bias)
                        nc.scalar.activation(
                            val[:], ps[:], mybir.ActivationFunctionType.Identity,
                            bias=b2c[:, t:t + 1], scale=1.0,
                        )
                    a_t = accs[t % NACC]
                    nc.vector.tensor_tensor(a_t[:], a_t[:], val[:], mybir.AluOpType.min)

            nc.vector.tensor_tensor(accs[0][:], accs[0][:], accs[1][:], mybir.AluOpType.min)
            nc.vector.tensor_tensor(accs[2][:], accs[2][:], accs[3][:], mybir.AluOpType.min)
            nc.vector.tensor_tensor(accs[0][:], accs[0][:], accs[2][:], mybir.AluOpType.min)

            mt = sb.tile([P, QB], mybir.dt.float32)
            for qb in range(QB):
                pst = psum2.tile([P, P], mybir.dt.float16, tag="pst")
                nc.tensor.transpose(pst[:], accs[0][:, qb * P:(qb + 1) * P], ident[:])
                nc.vector.tensor_reduce(
                    mt[:, qb:qb + 1], pst[:], mybir.AxisListType.X, mybir.AluOpType.min
                )
            nc.sync.dma_start(m_d[:], mt[:])

    nc.compile()
    return nc


def _get_nc():
    if "nc" not in _CACHE:
        _CACHE["nc"] = _build_nc()
    return _CACHE["nc"]


def kernel(mapped: np.ndarray, target: np.ndarray, indexes: np.ndarray) -> np.ndarray:
    from concourse.bass_utils import run_bass_kernel_spmd

    mapped = np.asarray(mapped, dtype=np.float32)
    target = np.asarray(target, dtype=np.float32)
    idx = np.asarray(indexes).astype(np.int64)

    # ---- host-side sharding / marshalling ----
    a = mapped[idx]                                   # [K, D]
    at_all = np.ascontiguousarray((-2.0 * a).T)       # [D, K]

    b2_64 = (target.astype(np.float64) ** 2).sum(1)   # exact fp64 row norms
    b2p = np.full(NYP, PADVAL + SHIFT, dtype=np.float64)
    b2p[:NY] = b2_64
    b2c_all = (b2p - SHIFT).astype(np.float32).reshape(TCH, P).T  # [P, TCH]

    tpad = np.zeros((NYP, D), dtype=np.float32)
    tpad[:NY] = target
    # tt[d_low, tc, dc, t] = target[tc*128 + t, dc*128 + d_low]
    tt_all = np.ascontiguousarray(
        tpad.reshape(TCH, P, DC, P).transpose(3, 0, 2, 1)
    ).astype(ml_dtypes.bfloat16)                       # [P, TCH, DC, P] bf16

    tt_half = [np.ascontiguousarray(tt_all[:, :TCHH]),
               np.ascontiguousarray(tt_all[:, TCHH:])]
    b2_half = [np.ascontiguousarray(b2c_all[:, :TCHH]),
               np.ascontiguousarray(b2c_all[:, TCHH:])]

    at_cores = []
    for cq in range(K // NQ):                          # 4 query slices
        at_cores.append(np.ascontiguousarray(
            at_all[:, cq * NQ:(cq + 1) * NQ].reshape(DC, P, NQ).transpose(1, 0, 2)
        ).astype(ml_dtypes.bfloat16))                  # [P, DC, NQ] bf16

    in_maps = []
    for c in range(NCORES):
        half = c // 4
        in_maps.append({"at": at_cores[c % 4], "tt": tt_half[half],
                        "b2c": b2_half[half]})

    # ---- run on the 8 NeuronCores ----
    nc = _get_nc()
    kwargs = {}
    if os.environ.get("KERNEL_TRACE_DIR"):
        kwargs["tmpdir"] = os.environ["KERNEL_TRACE_DIR"]
    res = run_bass_kernel_spmd(nc, in_maps, core_ids=list(range(NCORES)), **kwargs)
    _CACHE["last_res"] = res  # exec_time_ns/profile when BASS_TRACE=1

    # m[p, qb] on core c is query (c%4)*1024 + qb*128 + p, over target half c//4
    m_halves = [res.results[c]["m"].T.reshape(NQ) for c in range(NCORES)]
    m_dev = np.minimum(
        np.concatenate(m_halves[:4]), np.concatenate(m_halves[4:])
    ).astype(np.float64)                               # [K] shifted mins

    # ---- host decision + exact fallback ----
    t64 = None
    v = b2_64[idx] - 2.0 * np.einsum(
        "kd,kd->k", a.astype(np.float64), target[idx].astype(np.float64)
    ) - SHIFT                                          # shifted val at own index

    mismatch = m_dev < v - DELTA                       # confidently mismatched
    flagged = np.nonzero(~mismatch)[0]
    for q in flagged:
        if t64 is None:
            t64 = target.astype(np.float64)
        d2 = b2_64 - 2.0 * (t64 @ a[q].astype(np.float64))
        mismatch[q] = int(np.argmin(d2)) != int(idx[q])

    return np.asarray(mismatch.mean(), dtype=np.float32)


if __name__ == "__main__":
    rng = np.random.default_rng(1)
    mapped = rng.standard_normal((NX, D)).astype(np.float32)
    target = rng.standard_normal((NY, D)).astype(np.float32)
    indexes = rng.integers(0, NY, size=K).astype(np.int32)
    out = kernel(mapped=mapped, target=target, indexes=indexes)
    print("kernel output:", out, out.shape, out.dtype)


# revision 16
# speedup vs baseline: 1.1237x; 1.1237x over previous
"""Trainium2 8-core kernel for nn_AlignedGloveLayer (retrieval 1-NN mismatch loss).

Problem: a = mapped[indexes] ([4096, 256]); d2[k, j] = |a_k - target_j|^2 over
30000 targets; loss = mean over k of (argmin_j d2[k, j] != indexes[k]).

Only the comparison min_j d2 vs d2[:, indexes[k]] matters (sqrt is monotone and
the a2 term is constant per row), so the device computes, per query,
m_k = min_j (b2_j - 2 a_k . t_j). The mismatch decision and the final mean are
assembled on the host, with an exact fp64 fallback for any query whose margin
is within the device-arithmetic error bound (fp8 matmul + fp16 min).

Sharding (2x4 grid): cores 0-3 take 1024 queries each over the first half of
the targets; cores 4-7 take the same query slices over the second half; the
host mins the two halves. Per core, flipped orientation:
  psum[t, q] = sum_d T[t, d] * (-2 a[q, d])   (targets on psum partitions)
  ScalarE:  val16[t, q] = psum + (b2[t] - SHIFT)   (per-partition bias; a few
            chunks go through VectorE tensor_scalar instead to balance load)
  VectorE:  acc16 = min(acc16, val16)              (fp16, 2x mode, 4 rotating accs)
  final:    DMA the 4 fp16 accumulators out; host does the cross-partition min
Targets are padded 30000 -> 30720 = 240*128; padded rows get b2' = 60000.
"""
import os
import sys

for _p in ("/opt/trn_rl_repo", "/root/.axon_site/_ro/trn_rl_repo"):
    if os.path.isdir(_p) and _p not in sys.path:
        sys.path.append(_p)

from contextlib import ExitStack

import ml_dtypes
import numpy as np

NX, NY, D, K = 30000, 30000, 256, 4096
NCORES = 8
P = 128
DC = D // P          # 2 contraction chunks
NQ = 1024            # queries per core (cores c and c+4 share a query slice)
QB = NQ // P         # 8 query blocks
TCH = 240            # total target chunks: 240*128 = 30720 >= 30000
TCHH = TCH // 2      # target chunks per core (half of the targets)
NYP = TCH * P
GROUP = 4            # target chunks per DMA batch
DVE_AFFINE_EVERY = 6 # every Nth chunk's affine goes to VectorE instead of ScalarE
SHIFT = 512.0        # centers val16 in fp16 range
INIT = 60000.0       # min-accumulator init (> any real val')
PADVAL = 60000.0     # padded targets' shifted b2 (never the min)
DELTA = 18.0         # device error bound for host fallback flagging (fp8 matmul)

_CACHE: dict = {}


def _build_nc():
    import concourse.tile as tile
    from concourse import bacc, mybir
    nc = bacc.Bacc("TRN2", target_bir_lowering=False)
    at_d = nc.dram_tensor("at", [P, DC, NQ], mybir.dt.float8e4, kind="ExternalInput")
    tt_d = nc.dram_tensor("tt", [P, TCHH, DC, P], mybir.dt.float8e4, kind="ExternalInput")
    b2_d = nc.dram_tensor("b2c", [P, TCHH], mybir.dt.float32, kind="ExternalInput")
    m_d = nc.dram_tensor("m", [P, 8, NQ], mybir.dt.float16, kind="ExternalOutput")

    with tile.TileContext(nc) as tc:
        with ExitStack() as ctx:
            sb = ctx.enter_context(tc.tile_pool(name="sb", bufs=1))
            stream = ctx.enter_context(tc.tile_pool(name="stream", bufs=8))
            vals = ctx.enter_context(tc.tile_pool(name="vals", bufs=6))
            psum = ctx.enter_context(tc.tile_pool(name="psum", bufs=4, space="PSUM"))

            at = sb.tile([P, DC, NQ], mybir.dt.float8e4)
            nc.scalar.dma_start(at[:], at_d[:])
            b2c = sb.tile([P, TCHH], mybir.dt.float32)
            nc.scalar.dma_start(b2c[:], b2_d[:])
            NACC = 8     # rotating min accumulators (break DVE dependency chain)
            accs = []
            for i in range(NACC):
                a_t = sb.tile([P, NQ], mybir.dt.float16, tag=f"acc{i}", name=f"acc{i}")
                nc.gpsimd.memset(a_t[:], INIT)
                accs.append(a_t)

            for g in range(TCHH // GROUP):
                tt = stream.tile([P, GROUP, DC, P], mybir.dt.float8e4, tag="tt")
                nc.sync.dma_start(tt[:], tt_d[:, g * GROUP:(g + 1) * GROUP])
                for j in range(GROUP):
                    t = g * GROUP + j
                    ps = psum.tile([P, NQ], mybir.dt.float32)
                    # fp8 DoubleRow: full 256-deep contraction in one matmul,
                    # N<=512 psum limit -> two query halves
                    for h in range(NQ // 512):
                        nc.tensor.matmul(
                            ps[:, h * 512:(h + 1) * 512],
                            tt[:, j, :, :],
                            at[:, :, h * 512:(h + 1) * 512],
                            start=True, stop=True,
                            perf_mode=mybir.MatmulPerfMode.DoubleRow,
                        )
                    val = vals.tile([P, NQ], mybir.dt.float16, tag="val")
                    if t % DVE_AFFINE_EVERY == DVE_AFFINE_EVERY - 1:
                        # VectorE route (offload ScalarE): same affine
                        nc.vector.tensor_scalar(
                            val[:], ps[:], b2c[:, t:t + 1], None,
                            mybir.AluOpType.add,
                        )
                    else:
                        # ScalarE route: val = psum + b2' (per-partition bias)
                        nc.scalar.activation(
                            val[:], ps[:], mybir.ActivationFunctionType.Identity,
                            bias=b2c[:, t:t + 1], scale=1.0,
                        )
                    a_t = accs[t % NACC]
                    nc.vector.tensor_tensor(a_t[:], a_t[:], val[:], mybir.AluOpType.min)

            for i in range(NACC):
                nc.sync.dma_start(m_d[:, i], accs[i][:])

    nc.compile()
    return nc


def _get_nc():
    if "nc" not in _CACHE:
        _CACHE["nc"] = _build_nc()
    return _CACHE["nc"]


def kernel(mapped: np.ndarray, target: np.ndarray, indexes: np.ndarray) -> np.ndarray:
    from concourse.bass_utils import run_bass_kernel_spmd

    mapped = np.asarray(mapped, dtype=np.float32)
    target = np.asarray(target, dtype=np.float32)
    idx = np.asarray(indexes).astype(np.int64)

    # ---- host-side sharding / marshalling ----
    a = mapped[idx]                                   # [K, D]
    at_all = np.ascontiguousarray((-2.0 * a).T)       # [D, K]

    b2_64 = (target.astype(np.float64) ** 2).sum(1)   # exact fp64 row norms
    b2p = np.full(NYP, PADVAL + SHIFT, dtype=np.float64)
    b2p[:NY] = b2_64
    b2c_all = (b2p - SHIFT).astype(np.float32).reshape(TCH, P).T  # [P, TCH]

    tpad = np.zeros((NYP, D), dtype=np.float32)
    tpad[:NY] = target
    # tt[d_low, tc, dc, t] = target[tc*128 + t, dc*128 + d_low]
    tt_all = np.ascontiguousarray(
        tpad.reshape(TCH, P, DC, P).transpose(3, 0, 2, 1)
    ).astype(ml_dtypes.float8_e4m3)                    # [P, TCH, DC, P] fp8e4m3

    tt_half = [np.ascontiguousarray(tt_all[:, :TCHH]),
               np.ascontiguousarray(tt_all[:, TCHH:])]
    b2_half = [np.ascontiguousarray(b2c_all[:, :TCHH]),
               np.ascontiguousarray(b2c_all[:, TCHH:])]

    at_cores = []
    for cq in range(K // NQ):                          # 4 query slices
        at_cores.append(np.ascontiguousarray(
            at_all[:, cq * NQ:(cq + 1) * NQ].reshape(DC, P, NQ).transpose(1, 0, 2)
        ).astype(ml_dtypes.float8_e4m3))               # [P, DC, NQ] fp8e4m3

    in_maps = []
    for c in range(NCORES):
        half = c // 4
        in_maps.append({"at": at_cores[c % 4], "tt": tt_half[half],
                        "b2c": b2_half[half]})

    # ---- run on the 8 NeuronCores ----
    nc = _get_nc()
    kwargs = {}
    if os.environ.get("KERNEL_TRACE_DIR"):
        kwargs["tmpdir"] = os.environ["KERNEL_TRACE_DIR"]
    res = run_bass_kernel_spmd(nc, in_maps, core_ids=list(range(NCORES)), **kwargs)
    _CACHE["last_res"] = res  # exec_time_ns/profile when BASS_TRACE=1

    # m[p, i, q] on core c: acc i, target-partition p, query (c%4)*1024 + q;
    # min over p and i here, then across the two target halves
    m_halves = [res.results[c]["m"].min(axis=(0, 1)) for c in range(NCORES)]
    m_dev = np.minimum(
        np.concatenate(m_halves[:4]), np.concatenate(m_halves[4:])
    ).astype(np.float64)                               # [K] shifted mins

    # ---- host decision + exact fallback ----
    t64 = None
    v = b2_64[idx] - 2.0 * np.einsum(
        "kd,kd->k", a.astype(np.float64), target[idx].astype(np.float64)
    ) - SHIFT                                          # shifted val at own index

    mismatch = m_dev < v - DELTA                       # confidently mismatched
    flagged = np.nonzero(~mismatch)[0]
    for q in flagged:
        if t64 is None:
            t64 = target.astype(np.float64)
        d2 = b2_64 - 2.0 * (t64 @ a[q].astype(np.float64))
        mismatch[q] = int(np.argmin(d2)) != int(idx[q])

    return np.asarray(mismatch.mean(), dtype=np.float32)


if __name__ == "__main__":
    rng = np.random.default_rng(1)
    mapped = rng.standard_normal((NX, D)).astype(np.float32)
    target = rng.standard_normal((NY, D)).astype(np.float32)
    indexes = rng.integers(0, NY, size=K).astype(np.int32)
    out = kernel(mapped=mapped, target=target, indexes=indexes)
    print("kernel output:", out, out.shape, out.dtype)


# revision 17
# speedup vs baseline: 1.1270x; 1.0029x over previous
"""Trainium2 8-core kernel for nn_AlignedGloveLayer (retrieval 1-NN mismatch loss).

Problem: a = mapped[indexes] ([4096, 256]); d2[k, j] = |a_k - target_j|^2 over
30000 targets; loss = mean over k of (argmin_j d2[k, j] != indexes[k]).

Only the comparison min_j d2 vs d2[:, indexes[k]] matters (sqrt is monotone and
the a2 term is constant per row), so the device computes, per query,
m_k = min_j (b2_j - 2 a_k . t_j). The mismatch decision and the final mean are
assembled on the host, with an exact fp64 fallback for any query whose margin
is within the device-arithmetic error bound (fp8 matmul + fp16 min).

Sharding (2x4 grid): cores 0-3 take 1024 queries each over the first half of
the targets; cores 4-7 take the same query slices over the second half; the
host mins the two halves. Per core, flipped orientation:
  psum[t, q] = sum_d T[t, d] * (-2 a[q, d])   (targets on psum partitions)
  ScalarE:  val16[t, q] = psum + (b2[t] - SHIFT)   (per-partition bias; a few
            chunks go through VectorE tensor_scalar instead to balance load)
  VectorE:  acc16 = min(acc16, val16)              (fp16, 2x mode, 4 rotating accs)
  final:    DMA the 4 fp16 accumulators out; host does the cross-partition min
Targets are padded 30000 -> 30720 = 240*128; padded rows get b2' = 60000.
"""
import os
import sys

for _p in ("/opt/trn_rl_repo", "/root/.axon_site/_ro/trn_rl_repo"):
    if os.path.isdir(_p) and _p not in sys.path:
        sys.path.append(_p)

from contextlib import ExitStack

import ml_dtypes
import numpy as np

NX, NY, D, K = 30000, 30000, 256, 4096
NCORES = 8
P = 128
DC = D // P          # 2 contraction chunks
NQ = 1024            # queries per core (cores c and c+4 share a query slice)
QB = NQ // P         # 8 query blocks
TCH = 240            # total target chunks: 240*128 = 30720 >= 30000
TCHH = TCH // 2      # target chunks per core (half of the targets)
NYP = TCH * P
GROUP = 6            # target chunks per DMA batch
DVE_AFFINE_EVERY = 6 # every Nth chunk's affine goes to VectorE instead of ScalarE
SHIFT = 512.0        # centers val16 in fp16 range
INIT = 60000.0       # min-accumulator init (> any real val')
PADVAL = 60000.0     # padded targets' shifted b2 (never the min)
DELTA = 18.0         # device error bound for host fallback flagging (fp8 matmul)

_CACHE: dict = {}


def _build_nc():
    import concourse.tile as tile
    from concourse import bacc, mybir
    nc = bacc.Bacc("TRN2", target_bir_lowering=False)
    at_d = nc.dram_tensor("at", [P, DC, NQ], mybir.dt.float8e4, kind="ExternalInput")
    tt_d = nc.dram_tensor("tt", [P, TCHH, DC, P], mybir.dt.float8e4, kind="ExternalInput")
    b2_d = nc.dram_tensor("b2c", [P, TCHH], mybir.dt.float32, kind="ExternalInput")
    m_d = nc.dram_tensor("m", [P, 4, NQ], mybir.dt.float16, kind="ExternalOutput")

    with tile.TileContext(nc) as tc:
        with ExitStack() as ctx:
            sb = ctx.enter_context(tc.tile_pool(name="sb", bufs=1))
            stream = ctx.enter_context(tc.tile_pool(name="stream", bufs=8))
            vals = ctx.enter_context(tc.tile_pool(name="vals", bufs=6))
            psum = ctx.enter_context(tc.tile_pool(name="psum", bufs=4, space="PSUM"))

            at = sb.tile([P, DC, NQ], mybir.dt.float8e4)
            nc.scalar.dma_start(at[:], at_d[:])
            b2c = sb.tile([P, TCHH], mybir.dt.float32)
            nc.scalar.dma_start(b2c[:], b2_d[:])
            NACC = 4     # rotating min accumulators (break DVE dependency chain)
            accs = []
            for i in range(NACC):
                a_t = sb.tile([P, NQ], mybir.dt.float16, tag=f"acc{i}", name=f"acc{i}")
                nc.gpsimd.memset(a_t[:], INIT)
                accs.append(a_t)

            for g in range(TCHH // GROUP):
                tt = stream.tile([P, GROUP, DC, P], mybir.dt.float8e4, tag="tt")
                nc.sync.dma_start(tt[:], tt_d[:, g * GROUP:(g + 1) * GROUP])
                for j in range(GROUP):
                    t = g * GROUP + j
                    ps = psum.tile([P, NQ], mybir.dt.float32)
                    # fp8 DoubleRow: full 256-deep contraction in one matmul,
                    # N<=512 psum limit -> two query halves
                    for h in range(NQ // 512):
                        nc.tensor.matmul(
                            ps[:, h * 512:(h + 1) * 512],
                            tt[:, j, :, :],
                            at[:, :, h * 512:(h + 1) * 512],
                            start=True, stop=True,
                            perf_mode=mybir.MatmulPerfMode.DoubleRow,
                        )
                    val = vals.tile([P, NQ], mybir.dt.float16, tag="val")
                    if t % DVE_AFFINE_EVERY == DVE_AFFINE_EVERY - 1:
                        # VectorE route (offload ScalarE): same affine
                        nc.vector.tensor_scalar(
                            val[:], ps[:], b2c[:, t:t + 1], None,
                            mybir.AluOpType.add,
                        )
                    else:
                        # ScalarE route: val = psum + b2' (per-partition bias)
                        nc.scalar.activation(
                            val[:], ps[:], mybir.ActivationFunctionType.Identity,
                            bias=b2c[:, t:t + 1], scale=1.0,
                        )
                    a_t = accs[t % NACC]
                    nc.vector.tensor_tensor(a_t[:], a_t[:], val[:], mybir.AluOpType.min)

            for i in range(NACC):
                nc.sync.dma_start(m_d[:, i], accs[i][:])

    nc.compile()
    return nc


def _get_nc():
    if "nc" not in _CACHE:
        _CACHE["nc"] = _build_nc()
    return _CACHE["nc"]


def kernel(mapped: np.ndarray, target: np.ndarray, indexes: np.ndarray) -> np.ndarray:
    from concourse.bass_utils import run_bass_kernel_spmd

    mapped = np.asarray(mapped, dtype=np.float32)
    target = np.asarray(target, dtype=np.float32)
    idx = np.asarray(indexes).astype(np.int64)

    # ---- host-side sharding / marshalling ----
    a = mapped[idx]                                   # [K, D]
    at_all = np.ascontiguousarray((-2.0 * a).T)       # [D, K]

    b2_64 = (target.astype(np.float64) ** 2).sum(1)   # exact fp64 row norms
    b2p = np.full(NYP, PADVAL + SHIFT, dtype=np.float64)
    b2p[:NY] = b2_64
    b2c_all = (b2p - SHIFT).astype(np.float32).reshape(TCH, P).T  # [P, TCH]

    tpad = np.zeros((NYP, D), dtype=np.float32)
    tpad[:NY] = target
    # tt[d_low, tc, dc, t] = target[tc*128 + t, dc*128 + d_low]
    tt_all = np.ascontiguousarray(
        tpad.reshape(TCH, P, DC, P).transpose(3, 0, 2, 1)
    ).astype(ml_dtypes.float8_e4m3)                    # [P, TCH, DC, P] fp8e4m3

    tt_half = [np.ascontiguousarray(tt_all[:, :TCHH]),
               np.ascontiguousarray(tt_all[:, TCHH:])]
    b2_half = [np.ascontiguousarray(b2c_all[:, :TCHH]),
               np.ascontiguousarray(b2c_all[:, TCHH:])]

    at_cores = []
    for cq in range(K // NQ):                          # 4 query slices
        at_cores.append(np.ascontiguousarray(
            at_all[:, cq * NQ:(cq + 1) * NQ].reshape(DC, P, NQ).transpose(1, 0, 2)
        ).astype(ml_dtypes.float8_e4m3))               # [P, DC, NQ] fp8e4m3

    in_maps = []
    for c in range(NCORES):
        half = c // 4
        in_maps.append({"at": at_cores[c % 4], "tt": tt_half[half],
                        "b2c": b2_half[half]})

    # ---- run on the 8 NeuronCores ----
    nc = _get_nc()
    kwargs = {}
    if os.environ.get("KERNEL_TRACE_DIR"):
        kwargs["tmpdir"] = os.environ["KERNEL_TRACE_DIR"]
    res = run_bass_kernel_spmd(nc, in_maps, core_ids=list(range(NCORES)), **kwargs)
    _CACHE["last_res"] = res  # exec_time_ns/profile when BASS_TRACE=1

    # m[p, i, q] on core c: acc i, target-partition p, query (c%4)*1024 + q;
    # min over p and i here, then across the two target halves
    m_halves = [res.results[c]["m"].min(axis=(0, 1)) for c in range(NCORES)]
    m_dev = np.minimum(
        np.concatenate(m_halves[:4]), np.concatenate(m_halves[4:])
    ).astype(np.float64)                               # [K] shifted mins

    # ---- host decision + exact fallback ----
    t64 = None
    v = b2_64[idx] - 2.0 * np.einsum(
        "kd,kd->k", a.astype(np.float64), target[idx].astype(np.float64)
    ) - SHIFT                                          # shifted val at own index

    mismatch = m_dev < v - DELTA                       # confidently mismatched
    flagged = np.nonzero(~mismatch)[0]
    for q in flagged:
        if t64 is None:
            t64 = target.astype(np.float64)
        d2 = b2_64 - 2.0 * (t64 @ a[q].astype(np.float64))
        mismatch[q] = int(np.argmin(d2)) != int(idx[q])

    return np.asarray(mismatch.mean(), dtype=np.float32)


if __name__ == "__main__":
    rng = np.random.default_rng(1)
    mapped = rng.standard_normal((NX, D)).astype(np.float32)
    target = rng.standard_normal((NY, D)).astype(np.float32)
    indexes = rng.integers(0, NY, size=K).astype(np.int32)
    out = kernel(mapped=mapped, target=target, indexes=indexes)
    print("kernel output:", out, out.shape, out.dtype)
